# revision 16
# baseline (speedup 1.0000x reference)
"""Gaussian basis functions on 8 Trainium2 cores.

out[m] = sum_n w[n] * exp(-0.5 * (p_m - u_n)^T Sigma_n^{-1} (p_m - u_n))

Host precomputes per-Gaussian inverse covariance (O(N) tiny work), folds
log(w) into the exponent constant; the exponent becomes a single K=13
matmul:  exponent[m,n] = Paug[m,:] . Naug[n,:]
  Paug[m] = [pp(9), p(3), 1]                (built ON DEVICE from p)
  Naug[n] = [-0.5*A9, b, -0.5*uu + log w]   (per gaussian, host)
Each core gets M/8 = 8192 points (data parallel), N params replicated.

The axon tunnel to the remote TRN2 host has ~85-95ms RTT, so the host
path is organized for exactly one round trip:
  - upload raw centered points (3,MC) per core (786KB total) instead of
    the precomputed (13,MC) features (3.4MB); pp products built on device
  - output buffers for the bass_exec custom call are cached device-side
    (non-donating jit) so no per-call zeros upload
  - never block_until_ready before fetching: dispatch the execute, then
    immediately queue per-shard d2h copies -- they pipeline server-side
Device: build paug_r (13,MC) fp32r via 12 DMAs + one vector multiply,
then 64 m-tiles x 4 n-chunks of matmul(13x128x512) -> PSUM, scalar
engine Exp with accum_out (free-axis sum), one DMA of (128,64) result.
"""

import sys

sys.path.insert(0, "/opt/trn_rl_repo")

import numpy as np

M, N, NCORES = 65536, 2048, 8
MC = M // NCORES  # 8192 points per core
K = 13
MT = 128  # points per m-tile (PSUM partitions)
NT = 512  # gaussians per n-chunk (PSUM bank free size, f32)
NMT = MC // MT  # 64
NNT = N // NT  # 4
EPS_QUAT = 1e-8
EPS_COV = 1e-6

_CACHE = {}


def _build_bass():
    from concourse import bacc, tile
    import concourse.mybir as mybir
    from concourse.bass import MemorySpace

    f32 = mybir.dt.float32
    nc = bacc.Bacc(None, target_bir_lowering=False, debug=False)

    f16 = mybir.dt.float16
    pt_d = nc.dram_tensor("pt", [3, MC], f32, kind="ExternalInput")
    naug_d = nc.dram_tensor("naug", [K, N], f32, kind="ExternalInput")
    out_d = nc.dram_tensor("out", [MT, NMT], f16, kind="ExternalOutput")

    with tile.TileContext(nc) as tc:
        with (
            tc.tile_pool(name="const", bufs=1) as cpool,
            tc.tile_pool(name="work", bufs=2) as wpool,
            tc.tile_pool(name="psum", bufs=2, space=MemorySpace.PSUM) as ppool,
        ):
            f32r = mybir.dt.float32r
            # A rows: [p0 p1 p2 p0 p1 p2 p0 p1 p2 | p0 p1 p2 1]
            # B rows: [p0 p0 p0 p1 p1 p1 p2 p2 p2 | 1  1  1  1]
            # so A*B = paug: pp[3i+j]=p_i*p_j, then p, then 1 -- one
            # full-13-partition multiply (vector ops need partition
            # start 0; single-row memset at partition 12 is rejected,
            # so the ones rows come from a partition-0 memset tile via
            # SBUF-to-SBUF DMAs, which have no partition restrictions)
            A = cpool.tile([K, MC], f32)
            B = cpool.tile([K, MC], f32)
            ones = cpool.tile([1, MC], f32)
            naug = cpool.tile([K, N], f32)
            naug_r = cpool.tile([K, N], f32r)
            paug_r = cpool.tile([K, MC], f32r)
            acc = cpool.tile([MT, NMT], f32)
            acc16 = cpool.tile([MT, NMT], f16)
            nc.sync.dma_start(out=naug[:], in_=naug_d[:])
            nc.vector.memset(ones[:], 1.0)
            for r in range(4):  # rows 0-2,3-5,6-8 (pp factors) and 9-11 (p)
                nc.sync.dma_start(out=A[3 * r : 3 * r + 3, :], in_=pt_d[0:3, :])
            nc.sync.dma_start(out=A[12:13, :], in_=ones[:])
            for i in range(3):
                for j in range(3):
                    nc.sync.dma_start(
                        out=B[3 * i + j : 3 * i + j + 1, :], in_=pt_d[i : i + 1, :]
                    )
            for r in range(4):
                nc.sync.dma_start(out=B[9 + r : 10 + r, :], in_=ones[:])
            # PE fp32r mode needs inputs pre-rounded to the fp32r format;
            # vector-op output casts perform the rounding
            nc.vector.tensor_copy(naug_r[:], naug[:])
            nc.vector.tensor_mul(paug_r[:, :], A[:], B[:])

            for t in range(NMT):
                # full n-row per m-tile: (128, 2048) = 4 PSUM banks
                ps = ppool.tile([MT, N], f32, tag="ps")
                for j in range(NNT):
                    nc.tensor.matmul(
                        ps[:, j * NT : (j + 1) * NT],
                        paug_r[:, t * MT : (t + 1) * MT],
                        naug_r[:, j * NT : (j + 1) * NT],
                        start=True,
                        stop=True,
                    )
                scratch = wpool.tile([MT, N], f32, tag="scratch")
                nc.scalar.activation(
                    scratch[:],
                    ps[:],
                    mybir.ActivationFunctionType.Exp,
                    accum_out=acc[:, t : t + 1],
                )
            nc.vector.tensor_copy(acc16[:], acc[:])
            nc.sync.dma_start(out=out_d[:], in_=acc16[:])

    nc.compile()
    return nc


CENTER = 128.0  # volume center; shifting p and u leaves (p-u)^T A (p-u)
# unchanged but halves feature magnitudes (less cancellation in the PE)


def _prep_naug(positions, log_scales, rotations, weights):
    pos = positions.astype(np.float64) - CENTER
    s = np.exp(log_scales.astype(np.float64))
    q = rotations.astype(np.float64)
    q = q / (np.linalg.norm(q, axis=1, keepdims=True) + EPS_QUAT)
    w, x, y, z = q[:, 0], q[:, 1], q[:, 2], q[:, 3]
    R = np.empty((q.shape[0], 3, 3), np.float64)
    R[:, 0, 0] = 1 - 2 * (y * y + z * z)
    R[:, 0, 1] = 2 * (x * y - z * w)
    R[:, 0, 2] = 2 * (x * z + y * w)
    R[:, 1, 0] = 2 * (x * y + z * w)
    R[:, 1, 1] = 1 - 2 * (x * x + z * z)
    R[:, 1, 2] = 2 * (y * z - x * w)
    R[:, 2, 0] = 2 * (x * z - y * w)
    R[:, 2, 1] = 2 * (y * z + x * w)
    R[:, 2, 2] = 1 - 2 * (x * x + y * y)
    cov = np.einsum("nij,nj,nkj->nik", R, s * s, R) + EPS_COV * np.eye(3)
    Ainv = np.linalg.inv(cov)
    A9 = Ainv.reshape(-1, 9)
    b = np.einsum("nij,nj->ni", Ainv, pos)
    uu = np.einsum("ni,ni->n", pos, b)
    logw = np.log(np.maximum(weights.astype(np.float64), 1e-300))
    naug = np.concatenate(
        [-0.5 * A9, b, (-0.5 * uu + logw)[:, None]], axis=1
    ).T  # (13, N)
    return np.ascontiguousarray(naug).astype(np.float32)


def _prep_pt(points):
    # (M,3) -> per-core-concatenated (NCORES*3, MC), centered f32
    pc = points - np.float32(CENTER)
    pT = np.ascontiguousarray(pc.T)  # (3, M)
    return np.ascontiguousarray(
        pT.reshape(3, NCORES, MC).transpose(1, 0, 2).reshape(NCORES * 3, MC)
    )


def _get_runner():
    """Build the jitted shard_map executable once. Non-donating, so the
    bass_exec output operands can live device-side across calls."""
    if "runner" in _CACHE:
        return _CACHE["runner"]
    import jax
    from concourse import bass2jax
    from jax.sharding import Mesh, PartitionSpec, NamedSharding
    from jax.experimental.shard_map import shard_map
    import concourse.mybir as mybir

    nc = _CACHE.get("nc") or _build_bass()
    _CACHE["nc"] = nc
    bass2jax.install_neuronx_cc_hook()

    partition_name = nc.partition_id_tensor.name if nc.partition_id_tensor else None
    in_names, out_names, out_avals, zero_shapes = [], [], [], []
    for alloc in nc.m.functions[0].allocations:
        if not isinstance(alloc, mybir.MemoryLocationSet):
            continue
        name = alloc.memorylocations[0].name
        if alloc.kind == "ExternalInput":
            if name != partition_name:
                in_names.append(name)
        elif alloc.kind == "ExternalOutput":
            out_names.append(name)
            shape = tuple(alloc.tensor_shape)
            dtype = mybir.dt.np(alloc.dtype)
            out_avals.append(jax.core.ShapedArray(shape, dtype))
            zero_shapes.append((shape, dtype))
    n_params = len(in_names)
    all_names = list(in_names) + out_names
    if partition_name is not None:
        all_names.append(partition_name)

    def _body(*args):
        operands = list(args)
        if partition_name is not None:
            operands.append(bass2jax.partition_id_tensor())
        return tuple(
            bass2jax._bass_exec_p.bind(
                *operands,
                out_avals=tuple(out_avals),
                in_names=tuple(all_names),
                out_names=tuple(out_names),
                lowering_input_output_aliases=(),
                sim_require_finite=True,
                sim_require_nnan=True,
                nc=nc,
            )
        )

    devices = jax.devices()[:NCORES]
    mesh = Mesh(np.asarray(devices), ("core",))
    sh = NamedSharding(mesh, PartitionSpec("core"))
    repl = NamedSharding(mesh, PartitionSpec())
    n_outs = len(out_names)
    # naug is identical on every core: upload once to device 0 and let
    # the runtime replicate device-side (d2d), so only 106KB crosses
    # the tunnel instead of 8x that
    in_specs = tuple(
        PartitionSpec() if n == "naug" else PartitionSpec("core") for n in in_names
    ) + (PartitionSpec("core"),) * n_outs
    sharded = jax.jit(
        shard_map(
            _body,
            mesh=mesh,
            in_specs=in_specs,
            out_specs=(PartitionSpec("core"),) * n_outs,
            check_rep=False,
        ),
        keep_unused=True,
    )
    dev_zeros = [
        jax.device_put(np.zeros((NCORES * s[0], *s[1:]), d), sh)
        for (s, d) in zero_shapes
    ]
    jax.block_until_ready(dev_zeros)
    _CACHE["runner"] = (sharded, in_names, dev_zeros, sh, repl, devices)
    return _CACHE["runner"]


def kernel(points, positions, log_scales, rotations, weights):
    import jax

    sharded, in_names, dev_zeros, sh, repl, devices = _get_runner()

    # start the big upload first (async), overlap the naug host math
    pt_in = _prep_pt(points)
    dev_pt = jax.device_put(pt_in, sh)
    naug = _prep_naug(positions, log_scales, rotations, weights)
    dev_naug = jax.device_put(jax.device_put(naug, devices[0]), repl)
    inputs_by_name = {"pt": dev_pt, "naug": dev_naug}
    concat_in = [inputs_by_name[n] for n in in_names]

    out_arrs = sharded(*concat_in, *dev_zeros)
    # fetch without a prior block_until_ready: the d2h copies queue
    # behind the execute server-side (saves a full tunnel round trip)
    shards = sorted(
        out_arrs[0].addressable_shards, key=lambda s: s.index[0].start or 0
    )
    datas = [s.data for s in shards]
    for d in datas:
        d.copy_to_host_async()
    parts = [np.asarray(d) for d in datas]  # each (MT, NMT) f16
    arr = np.stack(parts, axis=0)  # (NCORES, MT, NMT)
    # out[c*MC + t*MT + p] = arr[c, p, t]
    return np.ascontiguousarray(arr.transpose(0, 2, 1)).reshape(-1).astype(np.float32)


# revision 17
# speedup vs baseline: 1.2045x; 1.2045x over previous
"""Gaussian basis functions on 8 Trainium2 cores.

out[m] = sum_n w[n] * exp(-0.5 * (p_m - u_n)^T Sigma_n^{-1} (p_m - u_n))

Host precomputes per-Gaussian inverse covariance (O(N) tiny work), folds
log(w) into the exponent constant; the exponent becomes a single K=13
matmul:  exponent[m,n] = Paug[m,:] . Naug[n,:]
  Paug[m] = [pp(9), p(3), 1]                (built ON DEVICE from p)
  Naug[n] = [-0.5*A9, b, -0.5*uu + log w]   (per gaussian, host)
Each core gets M/8 = 8192 points (data parallel), N params replicated.

The axon tunnel to the remote TRN2 host has ~85-95ms RTT, so the host
path is organized for exactly one round trip:
  - upload raw centered points (3,MC) per core (786KB total) instead of
    the precomputed (13,MC) features (3.4MB); pp products built on device
  - output buffers for the bass_exec custom call are cached device-side
    (non-donating jit) so no per-call zeros upload
  - never block_until_ready before fetching: dispatch the execute, then
    immediately queue per-shard d2h copies -- they pipeline server-side
Device: build paug_r (13,MC) fp32r via 12 DMAs + one vector multiply,
then 64 m-tiles x 4 n-chunks of matmul(13x128x512) -> PSUM, scalar
engine Exp with accum_out (free-axis sum), one DMA of (128,64) result.
"""

import sys

sys.path.insert(0, "/opt/trn_rl_repo")

import numpy as np

M, N, NCORES = 65536, 2048, 8
MC = M // NCORES  # 8192 points per core
K = 13
MT = 128  # points per m-tile (PSUM partitions)
NT = 512  # gaussians per n-chunk (PSUM bank free size, f32)
NMT = MC // MT  # 64
NNT = N // NT  # 4
EPS_QUAT = 1e-8
EPS_COV = 1e-6

_CACHE = {}


def _build_bass():
    from concourse import bacc, tile
    import concourse.mybir as mybir
    from concourse.bass import MemorySpace

    f32 = mybir.dt.float32
    nc = bacc.Bacc(None, target_bir_lowering=False, debug=False)

    f16 = mybir.dt.float16
    pt_d = nc.dram_tensor("pt", [3, MC], f32, kind="ExternalInput")
    naug_d = nc.dram_tensor("naug", [K, N], f32, kind="ExternalInput")
    out_d = nc.dram_tensor("out", [MT, NMT], f16, kind="ExternalOutput")

    with tile.TileContext(nc) as tc:
        with (
            tc.tile_pool(name="const", bufs=1) as cpool,
            tc.tile_pool(name="work", bufs=2) as wpool,
            tc.tile_pool(name="psum", bufs=2, space=MemorySpace.PSUM) as ppool,
        ):
            f32r = mybir.dt.float32r
            # A rows: [p0 p1 p2 p0 p1 p2 p0 p1 p2 | p0 p1 p2 1]
            # B rows: [p0 p0 p0 p1 p1 p1 p2 p2 p2 | 1  1  1  1]
            # so A*B = paug: pp[3i+j]=p_i*p_j, then p, then 1 -- one
            # full-13-partition multiply (vector ops need partition
            # start 0; single-row memset at partition 12 is rejected,
            # so the ones rows come from a partition-0 memset tile via
            # SBUF-to-SBUF DMAs, which have no partition restrictions)
            A = cpool.tile([K, MC], f32)
            B = cpool.tile([K, MC], f32)
            ones = cpool.tile([1, MC], f32)
            naug = cpool.tile([K, N], f32)
            naug_r = cpool.tile([K, N], f32r)
            paug_r = cpool.tile([K, MC], f32r)
            acc = cpool.tile([MT, NMT], f32)
            acc16 = cpool.tile([MT, NMT], f16)
            nc.sync.dma_start(out=naug[:], in_=naug_d[:])
            nc.vector.memset(ones[:], 1.0)
            for r in range(4):  # rows 0-2,3-5,6-8 (pp factors) and 9-11 (p)
                nc.sync.dma_start(out=A[3 * r : 3 * r + 3, :], in_=pt_d[0:3, :])
            nc.sync.dma_start(out=A[12:13, :], in_=ones[:])
            for i in range(3):
                for j in range(3):
                    nc.sync.dma_start(
                        out=B[3 * i + j : 3 * i + j + 1, :], in_=pt_d[i : i + 1, :]
                    )
            for r in range(4):
                nc.sync.dma_start(out=B[9 + r : 10 + r, :], in_=ones[:])
            # PE fp32r mode needs inputs pre-rounded to the fp32r format;
            # vector-op output casts perform the rounding
            nc.vector.tensor_copy(naug_r[:], naug[:])
            nc.vector.tensor_mul(paug_r[:, :], A[:], B[:])

            for t in range(NMT):
                # full n-row per m-tile: (128, 2048) = 4 PSUM banks
                ps = ppool.tile([MT, N], f32, tag="ps")
                for j in range(NNT):
                    nc.tensor.matmul(
                        ps[:, j * NT : (j + 1) * NT],
                        paug_r[:, t * MT : (t + 1) * MT],
                        naug_r[:, j * NT : (j + 1) * NT],
                        start=True,
                        stop=True,
                    )
                scratch = wpool.tile([MT, N], f32, tag="scratch")
                nc.scalar.activation(
                    scratch[:],
                    ps[:],
                    mybir.ActivationFunctionType.Exp,
                    accum_out=acc[:, t : t + 1],
                )
            nc.vector.tensor_copy(acc16[:], acc[:])
            nc.sync.dma_start(out=out_d[:], in_=acc16[:])

    nc.compile()
    return nc


CENTER = 128.0  # volume center; shifting p and u leaves (p-u)^T A (p-u)
# unchanged but halves feature magnitudes (less cancellation in the PE)


def _prep_naug(positions, log_scales, rotations, weights):
    pos = positions.astype(np.float64) - CENTER
    s = np.exp(log_scales.astype(np.float64))
    q = rotations.astype(np.float64)
    q = q / (np.linalg.norm(q, axis=1, keepdims=True) + EPS_QUAT)
    w, x, y, z = q[:, 0], q[:, 1], q[:, 2], q[:, 3]
    R = np.empty((q.shape[0], 3, 3), np.float64)
    R[:, 0, 0] = 1 - 2 * (y * y + z * z)
    R[:, 0, 1] = 2 * (x * y - z * w)
    R[:, 0, 2] = 2 * (x * z + y * w)
    R[:, 1, 0] = 2 * (x * y + z * w)
    R[:, 1, 1] = 1 - 2 * (x * x + z * z)
    R[:, 1, 2] = 2 * (y * z - x * w)
    R[:, 2, 0] = 2 * (x * z - y * w)
    R[:, 2, 1] = 2 * (y * z + x * w)
    R[:, 2, 2] = 1 - 2 * (x * x + y * y)
    cov = np.einsum("nij,nj,nkj->nik", R, s * s, R) + EPS_COV * np.eye(3)
    Ainv = np.linalg.inv(cov)
    A9 = Ainv.reshape(-1, 9)
    b = np.einsum("nij,nj->ni", Ainv, pos)
    uu = np.einsum("ni,ni->n", pos, b)
    logw = np.log(np.maximum(weights.astype(np.float64), 1e-300))
    naug = np.concatenate(
        [-0.5 * A9, b, (-0.5 * uu + logw)[:, None]], axis=1
    ).T  # (13, N)
    return np.ascontiguousarray(naug).astype(np.float32)


def _prep_pt(points):
    # (M,3) -> per-core-concatenated (NCORES*3, MC), centered f32
    pc = points - np.float32(CENTER)
    pT = np.ascontiguousarray(pc.T)  # (3, M)
    return np.ascontiguousarray(
        pT.reshape(3, NCORES, MC).transpose(1, 0, 2).reshape(NCORES * 3, MC)
    )


def _get_runner():
    """Build the jitted shard_map executable once. Non-donating, so the
    bass_exec output operands can live device-side across calls."""
    if "runner" in _CACHE:
        return _CACHE["runner"]
    import jax
    from concourse import bass2jax
    from jax.sharding import Mesh, PartitionSpec, NamedSharding
    from jax.experimental.shard_map import shard_map
    import concourse.mybir as mybir

    nc = _CACHE.get("nc") or _build_bass()
    _CACHE["nc"] = nc
    bass2jax.install_neuronx_cc_hook()

    partition_name = nc.partition_id_tensor.name if nc.partition_id_tensor else None
    in_names, out_names, out_avals, zero_shapes = [], [], [], []
    for alloc in nc.m.functions[0].allocations:
        if not isinstance(alloc, mybir.MemoryLocationSet):
            continue
        name = alloc.memorylocations[0].name
        if alloc.kind == "ExternalInput":
            if name != partition_name:
                in_names.append(name)
        elif alloc.kind == "ExternalOutput":
            out_names.append(name)
            shape = tuple(alloc.tensor_shape)
            dtype = mybir.dt.np(alloc.dtype)
            out_avals.append(jax.core.ShapedArray(shape, dtype))
            zero_shapes.append((shape, dtype))
    n_params = len(in_names)
    all_names = list(in_names) + out_names
    if partition_name is not None:
        all_names.append(partition_name)

    def _body(*args):
        operands = list(args)
        if partition_name is not None:
            operands.append(bass2jax.partition_id_tensor())
        return tuple(
            bass2jax._bass_exec_p.bind(
                *operands,
                out_avals=tuple(out_avals),
                in_names=tuple(all_names),
                out_names=tuple(out_names),
                lowering_input_output_aliases=(),
                sim_require_finite=True,
                sim_require_nnan=True,
                nc=nc,
            )
        )

    devices = jax.devices()[:NCORES]
    mesh = Mesh(np.asarray(devices), ("core",))
    sh = NamedSharding(mesh, PartitionSpec("core"))
    repl = NamedSharding(mesh, PartitionSpec())
    n_outs = len(out_names)
    # naug is identical on every core: upload once to device 0 and let
    # the runtime replicate device-side (d2d), so only 106KB crosses
    # the tunnel instead of 8x that
    in_specs = tuple(
        PartitionSpec() if n == "naug" else PartitionSpec("core") for n in in_names
    ) + (PartitionSpec("core"),) * n_outs
    sharded = jax.jit(
        shard_map(
            _body,
            mesh=mesh,
            in_specs=in_specs,
            out_specs=(PartitionSpec("core"),) * n_outs,
            check_rep=False,
        ),
        keep_unused=True,
    )
    dev_zeros = [
        jax.device_put(np.zeros((NCORES * s[0], *s[1:]), d), sh)
        for (s, d) in zero_shapes
    ]
    jax.block_until_ready(dev_zeros)
    _CACHE["runner"] = (sharded, in_names, dev_zeros, sh, repl, devices)
    return _CACHE["runner"]


def kernel(points, positions, log_scales, rotations, weights):
    import jax

    sharded, in_names, dev_zeros, sh, repl, devices = _get_runner()

    # start the big upload first (async), overlap the naug host math
    pt_in = _prep_pt(points)
    dev_pt = jax.device_put(pt_in, sh)
    naug = _prep_naug(positions, log_scales, rotations, weights)
    inputs_by_name = {"pt": dev_pt, "naug": naug}
    concat_in = [inputs_by_name[n] for n in in_names]

    out_arrs = sharded(*concat_in, *dev_zeros)
    # fetch without a prior block_until_ready: the d2h copies queue
    # behind the execute server-side (saves a full tunnel round trip)
    shards = sorted(
        out_arrs[0].addressable_shards, key=lambda s: s.index[0].start or 0
    )
    datas = [s.data for s in shards]
    for d in datas:
        d.copy_to_host_async()
    parts = [np.asarray(d) for d in datas]  # each (MT, NMT) f16
    arr = np.stack(parts, axis=0)  # (NCORES, MT, NMT)
    # out[c*MC + t*MT + p] = arr[c, p, t]
    return np.ascontiguousarray(arr.transpose(0, 2, 1)).reshape(-1).astype(np.float32)


# revision 18
# speedup vs baseline: 1.6109x; 1.3374x over previous
"""Gaussian basis functions on 8 Trainium2 cores.

out[m] = sum_n w[n] * exp(-0.5 * (p_m - u_n)^T Sigma_n^{-1} (p_m - u_n))

Host precomputes per-Gaussian inverse covariance (O(N) tiny work), folds
log(w) into the exponent constant; the exponent becomes a single K=13
matmul:  exponent[m,n] = Paug[m,:] . Naug[n,:]
  Paug[m] = [pp(9), p(3), 1]                (built ON DEVICE from p)
  Naug[n] = [-0.5*A9, b, -0.5*uu + log w]   (per gaussian, host)
Each core gets M/8 = 8192 points (data parallel), N params replicated.

The axon tunnel to the remote TRN2 host has ~85-95ms RTT, so the host
path is organized for exactly one round trip:
  - upload raw centered points (3,MC) per core (786KB total) instead of
    the precomputed (13,MC) features (3.4MB); pp products built on device
  - output buffers for the bass_exec custom call are cached device-side
    (non-donating jit) so no per-call zeros upload
  - never block_until_ready before fetching: dispatch the execute, then
    immediately queue per-shard d2h copies -- they pipeline server-side
Device: build paug_r (13,MC) fp32r via 12 DMAs + one vector multiply,
then 64 m-tiles x 4 n-chunks of matmul(13x128x512) -> PSUM, scalar
engine Exp with accum_out (free-axis sum), one DMA of (128,64) result.
"""

import sys

sys.path.insert(0, "/opt/trn_rl_repo")

import numpy as np

M, N, NCORES = 65536, 2048, 8
MC = M // NCORES  # 8192 points per core
K = 13
MT = 128  # points per m-tile (PSUM partitions)
NT = 512  # gaussians per n-chunk (PSUM bank free size, f32)
NMT = MC // MT  # 64
NNT = N // NT  # 4
EPS_QUAT = 1e-8
EPS_COV = 1e-6

_CACHE = {}


def _build_bass():
    from concourse import bacc, tile
    import concourse.mybir as mybir
    from concourse.bass import MemorySpace

    f32 = mybir.dt.float32
    nc = bacc.Bacc(None, target_bir_lowering=False, debug=False)

    f16 = mybir.dt.float16
    pt_d = nc.dram_tensor("pt", [3, MC], f32, kind="ExternalInput")
    naug_d = nc.dram_tensor("naug", [K, N], f32, kind="ExternalInput")
    out_d = nc.dram_tensor("out", [MT, NMT], f16, kind="ExternalOutput")

    with tile.TileContext(nc) as tc:
        with (
            tc.tile_pool(name="const", bufs=1) as cpool,
            tc.tile_pool(name="work", bufs=2) as wpool,
            tc.tile_pool(name="psum", bufs=2, space=MemorySpace.PSUM) as ppool,
        ):
            f32r = mybir.dt.float32r
            # A rows: [p0 p1 p2 p0 p1 p2 p0 p1 p2 | p0 p1 p2 1]
            # B rows: [p0 p0 p0 p1 p1 p1 p2 p2 p2 | 1  1  1  1]
            # so A*B = paug: pp[3i+j]=p_i*p_j, then p, then 1 -- one
            # full-13-partition multiply (vector ops need partition
            # start 0; single-row memset at partition 12 is rejected,
            # so the ones rows come from a partition-0 memset tile via
            # SBUF-to-SBUF DMAs, which have no partition restrictions)
            A = cpool.tile([K, MC], f32)
            B = cpool.tile([K, MC], f32)
            ones = cpool.tile([1, MC], f32)
            naug = cpool.tile([K, N], f32)
            naug_r = cpool.tile([K, N], f32r)
            paug_r = cpool.tile([K, MC], f32r)
            acc = cpool.tile([MT, NMT], f32)
            acc16 = cpool.tile([MT, NMT], f16)
            nc.sync.dma_start(out=naug[:], in_=naug_d[:])
            nc.vector.memset(ones[:], 1.0)
            for r in range(4):  # rows 0-2,3-5,6-8 (pp factors) and 9-11 (p)
                nc.sync.dma_start(out=A[3 * r : 3 * r + 3, :], in_=pt_d[0:3, :])
            nc.sync.dma_start(out=A[12:13, :], in_=ones[:])
            for i in range(3):
                for j in range(3):
                    nc.sync.dma_start(
                        out=B[3 * i + j : 3 * i + j + 1, :], in_=pt_d[i : i + 1, :]
                    )
            for r in range(4):
                nc.sync.dma_start(out=B[9 + r : 10 + r, :], in_=ones[:])
            # PE fp32r mode needs inputs pre-rounded to the fp32r format;
            # vector-op output casts perform the rounding
            nc.vector.tensor_copy(naug_r[:], naug[:])
            nc.vector.tensor_mul(paug_r[:, :], A[:], B[:])

            for t in range(NMT):
                # full n-row per m-tile: (128, 2048) = 4 PSUM banks
                ps = ppool.tile([MT, N], f32, tag="ps")
                for j in range(NNT):
                    nc.tensor.matmul(
                        ps[:, j * NT : (j + 1) * NT],
                        paug_r[:, t * MT : (t + 1) * MT],
                        naug_r[:, j * NT : (j + 1) * NT],
                        start=True,
                        stop=True,
                    )
                scratch = wpool.tile([MT, N], f32, tag="scratch")
                nc.scalar.activation(
                    scratch[:],
                    ps[:],
                    mybir.ActivationFunctionType.Exp,
                    accum_out=acc[:, t : t + 1],
                )
            nc.vector.tensor_copy(acc16[:], acc[:])
            nc.sync.dma_start(out=out_d[:], in_=acc16[:])

    nc.compile()
    return nc


CENTER = 128.0  # volume center; shifting p and u leaves (p-u)^T A (p-u)
# unchanged but halves feature magnitudes (less cancellation in the PE)


def _prep_naug(positions, log_scales, rotations, weights):
    pos = positions.astype(np.float64) - CENTER
    s = np.exp(log_scales.astype(np.float64))
    q = rotations.astype(np.float64)
    q = q / (np.linalg.norm(q, axis=1, keepdims=True) + EPS_QUAT)
    w, x, y, z = q[:, 0], q[:, 1], q[:, 2], q[:, 3]
    R = np.empty((q.shape[0], 3, 3), np.float64)
    R[:, 0, 0] = 1 - 2 * (y * y + z * z)
    R[:, 0, 1] = 2 * (x * y - z * w)
    R[:, 0, 2] = 2 * (x * z + y * w)
    R[:, 1, 0] = 2 * (x * y + z * w)
    R[:, 1, 1] = 1 - 2 * (x * x + z * z)
    R[:, 1, 2] = 2 * (y * z - x * w)
    R[:, 2, 0] = 2 * (x * z - y * w)
    R[:, 2, 1] = 2 * (y * z + x * w)
    R[:, 2, 2] = 1 - 2 * (x * x + y * y)
    cov = np.einsum("nij,nj,nkj->nik", R, s * s, R) + EPS_COV * np.eye(3)
    Ainv = np.linalg.inv(cov)
    A9 = Ainv.reshape(-1, 9)
    b = np.einsum("nij,nj->ni", Ainv, pos)
    uu = np.einsum("ni,ni->n", pos, b)
    logw = np.log(np.maximum(weights.astype(np.float64), 1e-300))
    naug = np.concatenate(
        [-0.5 * A9, b, (-0.5 * uu + logw)[:, None]], axis=1
    ).T  # (13, N)
    return np.ascontiguousarray(naug).astype(np.float32)


def _prep_pt(points):
    # (M,3) -> per-core-concatenated (NCORES*3, MC), centered f32
    pc = points - np.float32(CENTER)
    pT = np.ascontiguousarray(pc.T)  # (3, M)
    return np.ascontiguousarray(
        pT.reshape(3, NCORES, MC).transpose(1, 0, 2).reshape(NCORES * 3, MC)
    )


def _get_runner():
    """Build the jitted shard_map executable once. Non-donating, so the
    bass_exec output operands can live device-side across calls."""
    if "runner" in _CACHE:
        return _CACHE["runner"]
    import jax
    from concourse import bass2jax
    from jax.sharding import Mesh, PartitionSpec, NamedSharding
    from jax.experimental.shard_map import shard_map
    import concourse.mybir as mybir

    nc = _CACHE.get("nc") or _build_bass()
    _CACHE["nc"] = nc
    bass2jax.install_neuronx_cc_hook()

    partition_name = nc.partition_id_tensor.name if nc.partition_id_tensor else None
    in_names, out_names, out_avals, zero_shapes = [], [], [], []
    for alloc in nc.m.functions[0].allocations:
        if not isinstance(alloc, mybir.MemoryLocationSet):
            continue
        name = alloc.memorylocations[0].name
        if alloc.kind == "ExternalInput":
            if name != partition_name:
                in_names.append(name)
        elif alloc.kind == "ExternalOutput":
            out_names.append(name)
            shape = tuple(alloc.tensor_shape)
            dtype = mybir.dt.np(alloc.dtype)
            out_avals.append(jax.core.ShapedArray(shape, dtype))
            zero_shapes.append((shape, dtype))
    n_params = len(in_names)
    all_names = list(in_names) + out_names
    if partition_name is not None:
        all_names.append(partition_name)

    def _body(*args):
        operands = list(args)
        if partition_name is not None:
            operands.append(bass2jax.partition_id_tensor())
        return tuple(
            bass2jax._bass_exec_p.bind(
                *operands,
                out_avals=tuple(out_avals),
                in_names=tuple(all_names),
                out_names=tuple(out_names),
                lowering_input_output_aliases=(),
                sim_require_finite=True,
                sim_require_nnan=True,
                nc=nc,
            )
        )

    devices = jax.devices()[:NCORES]
    mesh = Mesh(np.asarray(devices), ("core",))
    sh = NamedSharding(mesh, PartitionSpec("core"))
    n_outs = len(out_names)
    sharded = jax.jit(
        shard_map(
            _body,
            mesh=mesh,
            in_specs=(PartitionSpec("core"),) * (n_params + n_outs),
            out_specs=(PartitionSpec("core"),) * n_outs,
            check_rep=False,
        ),
        keep_unused=True,
    )
    dev_zeros = [
        jax.device_put(np.zeros((NCORES * s[0], *s[1:]), d), sh)
        for (s, d) in zero_shapes
    ]
    jax.block_until_ready(dev_zeros)
    _CACHE["runner"] = (sharded, in_names, dev_zeros, sh)
    return _CACHE["runner"]


def kernel(points, positions, log_scales, rotations, weights):
    import jax

    sharded, in_names, dev_zeros, sh = _get_runner()

    # start the big upload first (async), overlap the naug host math
    pt_in = _prep_pt(points)
    dev_pt = jax.device_put(pt_in, sh)
    naug = _prep_naug(positions, log_scales, rotations, weights)
    inputs_by_name = {"pt": dev_pt, "naug": np.tile(naug, (NCORES, 1))}
    concat_in = [inputs_by_name[n] for n in in_names]

    out_arrs = sharded(*concat_in, *dev_zeros)
    # fetch without a prior block_until_ready: the d2h copies queue
    # behind the execute server-side (saves a full tunnel round trip)
    shards = sorted(
        out_arrs[0].addressable_shards, key=lambda s: s.index[0].start or 0
    )
    datas = [s.data for s in shards]
    for d in datas:
        d.copy_to_host_async()
    parts = [np.asarray(d) for d in datas]  # each (MT, NMT) f16
    arr = np.stack(parts, axis=0)  # (NCORES, MT, NMT)
    # out[c*MC + t*MT + p] = arr[c, p, t]
    return np.ascontiguousarray(arr.transpose(0, 2, 1)).reshape(-1).astype(np.float32)
